# revision 9
# baseline (speedup 1.0000x reference)
"""Chamfer distance (L2) Bass kernel for 8 TRN2 NeuronCores.

Problem: xyz1 [B=8, N=8192, 3] f32, xyz2 [B=8, M=8192, 3] f32.
  d[b, n, m] = |xyz1[b,n] - xyz2[b,m]|^2
  dist1[b, n] = min_m d[b, n, m];  dist2[b, m] = min_n d[b, n, m]

Sharding: data-parallel over batch — core b handles batch b (B == n_cores == 8).
No collectives; outputs are gathered on the host.

Per-core algorithm — single pass over the distance matrix:
  d = x2[n] + y2[m] - 2*x.y is produced tile-by-tile by ONE TensorE matmul per
  output chunk via an augmented K=13 bf16 contraction (hi/lo splits of the
  coordinates for ~fp16-accurate cross terms, ones-rows folding the squared
  norms in), landing in fp32 PSUM groups of [128, 2048].

  Row-tiles are STAGED: ScalarE (the cheap PSUM reader) copies the four
  PSUM groups into one [128, 8192] fp16 SBUF tile cp, then per row-tile:
    - colmin:  acc2 = min(acc2, cp)            (one wide DVE tensor_tensor,
               2x all-16-bit mode; serial chain through acc2)
    - rowmin:  a log2 fold chain of in-place halving TT mins
               8192->4096->2048->1024->512 + one 1x tensor_reduce. The first
               (most expensive) fold runs on GPSIMD for most tiles, which
               balances the three engines; the DVE finishes the tail.
  (tensor_tensor_reduce would fuse the fold+reduce, but that ISA op is
  broken on this runtime — it wedges the NeuronCore.)

  dist1 comes straight from rowp; dist2's final min over the 128 partition
  lanes of acc2 uses PE transposes of 128x128 blocks + free-axis reduce_min.
"""

import sys

if "/opt/trn_rl_repo" not in sys.path:
    sys.path.insert(0, "/opt/trn_rl_repo")

import numpy as np
import ml_dtypes

import concourse.bass as bass  # noqa: F401
import concourse.mybir as mybir
import concourse.tile as tile
from concourse import bacc
from concourse.bass_utils import run_bass_kernel_spmd
from concourse.masks import make_identity

BF16 = ml_dtypes.bfloat16

B = 8
N = 8192
M = 8192
P = 128  # output rows per tile (partition dim)
K = 13  # augmented contraction rows
BIG = 60000.0  # min-identity; finite in fp16, >> any squared distance here
ST = "float16"  # staging/accumulator dtype: 16-bit for DVE 2x mode, 2^-11 rounding

_NC_CACHE = {}


def _emit_transposed(tc, nc, pool, vec_sb, ident, out_dram):
    """vec_sb [P, n_blk] fp16 holds out[i*P + p] at [p, i]. PE-transpose to
    [n_blk, P], cast-copy to fp32, and DMA out contiguously (the direct
    [p, i]-strided DMA would scatter 4-byte elements)."""
    n_blk = vec_sb.shape[1]
    st = getattr(mybir.dt, ST)
    with tc.tile_pool(name="psum_o", bufs=1, space="PSUM") as psum_o:
        pt = psum_o.tile([n_blk, P], st)
        nc.tensor.transpose(pt[:, :], vec_sb[:, :], ident[:, :])
        ot = pool.tile([n_blk, P], mybir.dt.float32, tag="out_t")
        nc.vector.tensor_copy(ot[:, :], pt[:, :])
        nc.sync.dma_start(
            out=out_dram.ap().rearrange("(i p) -> i p", p=P), in_=ot[:, :]
        )


def _part_min_out(tc, nc, pool, acc, ident, out_dram):
    """Min over the 128 partitions of acc -> [m] via PE transpose + reduce."""
    m_len = acc.shape[1]
    n_blk = m_len // P
    st = getattr(mybir.dt, ST)
    osb = pool.tile([P, n_blk], st, tag="partmin_out")
    with tc.tile_pool(name="psum_t", bufs=2, space="PSUM") as psum_t:
        for t in range(n_blk):
            pst = psum_t.tile([P, P], st)
            nc.tensor.transpose(pst[:, :], acc[:, t * P : (t + 1) * P], ident[:, :])
            nc.vector.tensor_reduce(
                out=osb[:, t : t + 1],
                in_=pst[:, :],
                axis=mybir.AxisListType.X,
                op=mybir.AluOpType.min,
            )
    _emit_transposed(tc, nc, pool, osb, ident, out_dram)


def build_nc(n, m, mm_free=512, ps_group=2048, reps=1, direct_mod=0, gps_8=0):
    """Build + compile the per-core Bass program (SPMD, same on all cores).

    direct_mod>0: every direct_mod-th row-tile skips ScalarE staging and is
    consumed from PSUM by the DVE (1x mode) — balances ACT vs DVE load.
    gps_8: of every 8 row-tiles, this many get their first rowmin fold done
    by GPSIMD instead of the DVE (engine load balancing).
    reps>1 repeats the main pass (identical results — min is idempotent);
    used only for timing: kernel time = slope of wall time vs reps.
    """
    ps_group = min(ps_group, m)
    mm_free = min(mm_free, ps_group)
    st = getattr(mybir.dt, ST)
    n_tiles = n // P
    n_groups = m // ps_group
    half = m // 2
    ghalf = ps_group // 2

    nc = bacc.Bacc("TRN2", target_bir_lowering=False, debug=False)
    sx = nc.dram_tensor("sx", [K, n], mybir.dt.bfloat16, kind="ExternalInput")
    my = nc.dram_tensor("my", [K, m], mybir.dt.bfloat16, kind="ExternalInput")
    d1 = nc.dram_tensor("dist1", [n], mybir.dt.float32, kind="ExternalOutput")
    d2 = nc.dram_tensor("dist2", [m], mybir.dt.float32, kind="ExternalOutput")

    with tile.TileContext(nc) as tc:
        with tc.tile_pool(name="singles", bufs=1) as singles:
            sx_sb = singles.tile([K, n], mybir.dt.bfloat16)
            my_sb = singles.tile([K, m], mybir.dt.bfloat16)
            nc.sync.dma_start(out=sx_sb[:, :], in_=sx.ap())
            nc.sync.dma_start(out=my_sb[:, :], in_=my.ap())

            acc2 = singles.tile([P, m], st)
            rowp = singles.tile([P, n_tiles], st)
            nc.vector.memset(acc2[:, :], BIG)

            with (
                tc.tile_pool(name="psum", bufs=2, space="PSUM") as psum_pool,
                tc.tile_pool(name="cp", bufs=2) as cp_pool,
                tc.tile_pool(name="scr", bufs=2) as scr_pool,
            ):
                import contextlib

                rep_ctx = (
                    tc.For_i(0, reps, 1) if reps > 1 else contextlib.nullcontext()
                )
                with rep_ctx:
                  for i in range(n_tiles):
                    lhsT = sx_sb[:, i * P : (i + 1) * P]
                    direct = direct_mod > 0 and i % direct_mod == 0
                    cp = (
                        None
                        if direct
                        else cp_pool.tile([P, m], st, tag="cp")
                    )
                    rt = (
                        scr_pool.tile([P, n_groups], st, tag="rtmp")
                        if direct
                        else None
                    )
                    for g in range(n_groups):
                        ps = psum_pool.tile([P, ps_group], mybir.dt.float32)
                        for t in range(ps_group // mm_free):
                            lo = g * ps_group + t * mm_free
                            nc.tensor.matmul(
                                ps[:, t * mm_free : (t + 1) * mm_free],
                                lhsT=lhsT,
                                rhs=my_sb[:, lo : lo + mm_free],
                                start=True,
                                stop=True,
                            )
                        if direct:
                            # DVE consumes PSUM at 1x — no ScalarE involved
                            sl = acc2[:, g * ps_group : (g + 1) * ps_group]
                            nc.vector.tensor_tensor(
                                out=sl, in0=ps[:, :], in1=sl, op=mybir.AluOpType.min
                            )
                            nc.vector.tensor_reduce(
                                out=rt[:, g : g + 1],
                                in_=ps[:, :],
                                axis=mybir.AxisListType.X,
                                op=mybir.AluOpType.min,
                            )
                            if g == n_groups - 1:
                                nc.vector.tensor_reduce(
                                    out=rowp[:, i : i + 1],
                                    in_=rt[:, :],
                                    axis=mybir.AxisListType.X,
                                    op=mybir.AluOpType.min,
                                )
                        else:
                            nc.scalar.copy(
                                out=cp[:, g * ps_group : (g + 1) * ps_group],
                                in_=ps[:, :],
                            )
                    if not direct:
                        # colmin over the whole staged row-tile, one wide op
                        nc.vector.tensor_tensor(
                            out=acc2[:, :],
                            in0=cp[:, :],
                            in1=acc2[:, :],
                            op=mybir.AluOpType.min,
                        )
                        # rowmin: halving fold chain, in place in scr
                        scr = scr_pool.tile([P, half], st, tag="scr")
                        eng0 = nc.gpsimd if (i % 8) < gps_8 else nc.vector
                        eng0.tensor_tensor(
                            out=scr[:, :],
                            in0=cp[:, :half],
                            in1=cp[:, half:],
                            op=mybir.AluOpType.min,
                        )
                        w = half // 2
                        while w >= 512:
                            nc.vector.tensor_tensor(
                                out=scr[:, :w],
                                in0=scr[:, :w],
                                in1=scr[:, w : 2 * w],
                                op=mybir.AluOpType.min,
                            )
                            w //= 2
                        nc.vector.tensor_reduce(
                            out=rowp[:, i : i + 1],
                            in_=scr[:, : 2 * w],
                            axis=mybir.AxisListType.X,
                            op=mybir.AluOpType.min,
                        )

            ident = singles.tile([P, P], st)
            make_identity(nc, ident[:, :])
            _emit_transposed(tc, nc, singles, rowp, ident, d1)
            _part_min_out(tc, nc, singles, acc2, ident, d2)

    nc.compile()
    return nc


def get_nc(n=N, m=M, reps=1, **kw):
    key = (n, m, reps, tuple(sorted(kw.items())))
    if key not in _NC_CACHE:
        _NC_CACHE[key] = build_nc(n, m, reps=reps, **kw)
    return _NC_CACHE[key]


def _split_hi_lo(a):
    hi = a.astype(BF16)
    lo = (a - hi.astype(np.float32)).astype(BF16)
    return hi, lo


def _stat_rows(u, u2h, u2l):
    """Stationary-side augmented rows [13, len] for points u [len, 3] f32."""
    uh, ul = _split_hi_lo(u)
    out = np.empty((K, u.shape[0]), BF16)
    out[0:3] = uh.T
    out[3:6] = uh.T
    out[6:9] = ul.T
    out[9] = BF16(1.0)
    out[10] = BF16(1.0)
    out[11] = u2h
    out[12] = u2l
    return np.ascontiguousarray(out)


def _mov_rows(v, v2h, v2l):
    """Moving-side augmented rows [13, len] for points v [len, 3] f32."""
    vh, vl = _split_hi_lo(v)
    out = np.empty((K, v.shape[0]), BF16)
    out[0:3] = (-2.0 * vh.astype(np.float32)).astype(BF16).T
    out[3:6] = (-2.0 * vl.astype(np.float32)).astype(BF16).T
    out[6:9] = out[0:3]
    out[9] = v2h
    out[10] = v2l
    out[11] = BF16(1.0)
    out[12] = BF16(1.0)
    return np.ascontiguousarray(out)


def _prep_core_inputs(x, y):
    """Augmented bf16 matrices for one batch: core computes d[n-tile, m] tiles
    with x stationary and y moving; both reductions happen in the same pass."""
    x = np.asarray(x, np.float32)
    y = np.asarray(y, np.float32)
    x2 = np.sum(x.astype(np.float64) * x, axis=-1).astype(np.float32)
    y2 = np.sum(y.astype(np.float64) * y, axis=-1).astype(np.float32)
    x2h, x2l = _split_hi_lo(x2)
    y2h, y2l = _split_hi_lo(y2)
    return {
        "sx": _stat_rows(x, x2h, x2l),
        "my": _mov_rows(y, y2h, y2l),
    }


def kernel(xyz1, xyz2):
    xyz1 = np.asarray(xyz1, np.float32)
    xyz2 = np.asarray(xyz2, np.float32)
    b, n, _ = xyz1.shape
    m = xyz2.shape[1]
    assert b == B and n == N and m == M, (b, n, m)

    nc = get_nc(n, m)
    in_maps = [_prep_core_inputs(xyz1[i], xyz2[i]) for i in range(b)]
    res = run_bass_kernel_spmd(nc, in_maps, core_ids=list(range(b)))
    dist1 = np.stack([res.results[i]["dist1"] for i in range(b)]).astype(np.float32)
    dist2 = np.stack([res.results[i]["dist2"] for i in range(b)]).astype(np.float32)
    return dist1, dist2


# revision 16
# speedup vs baseline: 1.5714x; 1.5714x over previous
"""Chamfer distance (L2) Bass kernel for 8 TRN2 NeuronCores.

Problem: xyz1 [B=8, N=8192, 3] f32, xyz2 [B=8, M=8192, 3] f32.
  d[b, n, m] = |xyz1[b,n] - xyz2[b,m]|^2
  dist1[b, n] = min_m d[b, n, m];  dist2[b, m] = min_n d[b, n, m]

Sharding: data-parallel over batch — core b handles batch b (B == n_cores == 8).
No collectives; outputs are gathered on the host.

Per-core algorithm — single pass over the distance matrix:
  d = x2[n] + y2[m] - 2*x.y is produced tile-by-tile by ONE TensorE matmul per
  output chunk via an augmented K=13 bf16 contraction (hi/lo splits of the
  coordinates for ~fp16-accurate cross terms, ones-rows folding the squared
  norms in), landing in fp32 PSUM groups of [128, 2048].

  Row-tiles are STAGED: ScalarE (the cheap PSUM reader) copies the four
  PSUM groups into one [128, 8192] fp16 SBUF tile cp, then per row-tile:
    - colmin:  acc2 = min(acc2, cp)            (one wide DVE tensor_tensor,
               2x all-16-bit mode; serial chain through acc2)
    - rowmin:  a log2 fold chain of in-place halving TT mins
               8192->4096->2048->1024->512 + one 1x tensor_reduce. The first
               (most expensive) fold runs on GPSIMD for most tiles, which
               balances the three engines; the DVE finishes the tail.
  (tensor_tensor_reduce would fuse the fold+reduce, but that ISA op is
  broken on this runtime — it wedges the NeuronCore.)

  dist1 comes straight from rowp; dist2's final min over the 128 partition
  lanes of acc2 uses PE transposes of 128x128 blocks + free-axis reduce_min.
"""

import sys

if "/opt/trn_rl_repo" not in sys.path:
    sys.path.insert(0, "/opt/trn_rl_repo")

import numpy as np
import ml_dtypes

import concourse.bass as bass  # noqa: F401
import concourse.mybir as mybir
import concourse.tile as tile
from concourse import bacc
from concourse.bass_utils import run_bass_kernel_spmd
from concourse.masks import make_identity


def _register_minmin_reduce():
    """Register a custom DVE op: out = min(in0, in1); accum_out =
    min(s0, min_k out[k]).  Fuses the rowmin fold layer-1 with the free-axis
    reduce — one DVE pass instead of a 4-op fold chain + reduce.  The op is
    appended to concourse.dve_ops.OPS at import (the per-NEFF DVE table is
    generated from that registry); the sha pin is self-computed."""
    import numpy as _np
    import concourse.dve_ops as dve_ops
    from concourse.dve_spec import C0, Spec, Src0, Src1, lower, minn
    from concourse.dve_uop import DveOpSpec

    name = "TT_MIN_MIN_REDUCE_ANT"
    for o in dve_ops.OPS:
        if o.name == name:
            return o

    def _ref(in0, in1, c0, c1, c2):
        b = _np.minimum(in0.astype(_np.float32), in1.astype(_np.float32))
        a = _np.minimum(
            _np.float32(c0), b.reshape(b.shape[0], -1).min(axis=-1, keepdims=True)
        )
        return b, a

    spec = Spec(body=minn(Src0, Src1), accum=minn, accum_init=C0, reference=_ref)
    row = max(dve_ops._SUB_OPCODE_FOR_NAME.values()) + 1
    assert row < 0x20
    shas = {}
    for ver in ("v3",):
        uops = lower(spec, ver=ver)
        shas[ver] = DveOpSpec(name=name, opcode=row, uops=uops, rd1_en=True).sha(ver)
    op = dve_ops.DveOp(name, spec, subdim=False, uops_sha=shas)
    dve_ops.OPS.append(op)
    dve_ops.CUSTOM_DVE_SPECS[name] = spec
    dve_ops._SUB_OPCODE_FOR_NAME[name] = row
    return op


try:
    _MINMIN = _register_minmin_reduce()
except Exception:  # pragma: no cover — fall back to the stock fold chain
    _MINMIN = None

BF16 = ml_dtypes.bfloat16

B = 8
N = 8192
M = 8192
P = 128  # output rows per tile (partition dim)
K = 13  # augmented contraction rows
BIG = 60000.0  # min-identity; finite in fp16, >> any squared distance here
ST = "float16"  # staging/accumulator dtype: 16-bit for DVE 2x mode, 2^-11 rounding

_NC_CACHE = {}


def _emit_transposed(tc, nc, pool, vec_sb, ident, out_dram):
    """vec_sb [P, n_blk] fp16 holds out[i*P + p] at [p, i]. PE-transpose to
    [n_blk, P], cast-copy to fp32, and DMA out contiguously (the direct
    [p, i]-strided DMA would scatter 4-byte elements)."""
    n_blk = vec_sb.shape[1]
    st = getattr(mybir.dt, ST)
    with tc.tile_pool(name="psum_o", bufs=1, space="PSUM") as psum_o:
        pt = psum_o.tile([n_blk, P], st)
        nc.tensor.transpose(pt[:, :], vec_sb[:, :], ident[:, :])
        ot = pool.tile([n_blk, P], mybir.dt.float32, tag="out_t")
        nc.vector.tensor_copy(ot[:, :], pt[:, :])
        nc.sync.dma_start(
            out=out_dram.ap().rearrange("(i p) -> i p", p=P), in_=ot[:, :]
        )


def _part_min_out(tc, nc, pool, acc, ident, out_dram):
    """Min over the 128 partitions of acc -> [m] via PE transpose + reduce."""
    m_len = acc.shape[1]
    n_blk = m_len // P
    st = getattr(mybir.dt, ST)
    osb = pool.tile([P, n_blk], st, tag="partmin_out")
    with tc.tile_pool(name="psum_t", bufs=2, space="PSUM") as psum_t:
        for t in range(n_blk):
            pst = psum_t.tile([P, P], st)
            nc.tensor.transpose(pst[:, :], acc[:, t * P : (t + 1) * P], ident[:, :])
            nc.vector.tensor_reduce(
                out=osb[:, t : t + 1],
                in_=pst[:, :],
                axis=mybir.AxisListType.X,
                op=mybir.AluOpType.min,
            )
    _emit_transposed(tc, nc, pool, osb, ident, out_dram)


def build_nc(
    n,
    m,
    mm_free=512,
    ps_group=2048,
    reps=1,
    direct_mod=0,
    gps_8=0,
    cp_bufs=2,
    scr_bufs=2,
    psum_bufs=2,
    fine=0,
    fused=1,
):
    """Build + compile the per-core Bass program (SPMD, same on all cores).

    direct_mod>0: every direct_mod-th row-tile skips ScalarE staging and is
    consumed from PSUM by the DVE (1x mode) — balances ACT vs DVE load.
    gps_8: of every 8 row-tiles, this many get their first rowmin fold done
    by GPSIMD instead of the DVE (engine load balancing).
    fine=1: per-PSUM-group colmin/fold ops instead of tile-wide ops.
    reps>1 repeats the main pass (identical results — min is idempotent);
    used only for timing: kernel time = slope of wall time vs reps.
    """
    ps_group = min(ps_group, m)
    mm_free = min(mm_free, ps_group)
    st = getattr(mybir.dt, ST)
    n_tiles = n // P
    n_groups = m // ps_group
    half = m // 2
    ghalf = ps_group // 2

    nc = bacc.Bacc("TRN2", target_bir_lowering=False, debug=False)
    sx = nc.dram_tensor("sx", [K, n], mybir.dt.bfloat16, kind="ExternalInput")
    my = nc.dram_tensor("my", [K, m], mybir.dt.bfloat16, kind="ExternalInput")
    d1 = nc.dram_tensor("dist1", [n], mybir.dt.float32, kind="ExternalOutput")
    d2 = nc.dram_tensor("dist2", [m], mybir.dt.float32, kind="ExternalOutput")

    with tile.TileContext(nc) as tc:
        with tc.tile_pool(name="singles", bufs=1) as singles:
            sx_sb = singles.tile([K, n], mybir.dt.bfloat16)
            my_sb = singles.tile([K, m], mybir.dt.bfloat16)
            nc.sync.dma_start(out=sx_sb[:, :], in_=sx.ap())
            nc.sync.dma_start(out=my_sb[:, :], in_=my.ap())

            acc2 = singles.tile([P, m], st)
            rowp = singles.tile([P, n_tiles], st)
            nc.vector.memset(acc2[:, :], BIG)

            with (
                tc.tile_pool(name="psum", bufs=psum_bufs, space="PSUM") as psum_pool,
                tc.tile_pool(name="cp", bufs=cp_bufs) as cp_pool,
                tc.tile_pool(name="scr", bufs=scr_bufs) as scr_pool,
            ):
                import contextlib

                rep_ctx = (
                    tc.For_i(0, reps, 1) if reps > 1 else contextlib.nullcontext()
                )
                with rep_ctx:
                  for i in range(n_tiles):
                    lhsT = sx_sb[:, i * P : (i + 1) * P]
                    direct = direct_mod > 0 and i % direct_mod == 0
                    cp = (
                        None
                        if direct
                        else cp_pool.tile([P, m], st, tag="cp")
                    )
                    rt = (
                        scr_pool.tile([P, n_groups], st, tag="rtmp")
                        if direct
                        else None
                    )
                    scr = None
                    for g in range(n_groups):
                        ps = psum_pool.tile([P, ps_group], mybir.dt.float32)
                        for t in range(ps_group // mm_free):
                            lo = g * ps_group + t * mm_free
                            nc.tensor.matmul(
                                ps[:, t * mm_free : (t + 1) * mm_free],
                                lhsT=lhsT,
                                rhs=my_sb[:, lo : lo + mm_free],
                                start=True,
                                stop=True,
                            )
                        if direct:
                            # DVE consumes PSUM at 1x — no ScalarE involved
                            sl = acc2[:, g * ps_group : (g + 1) * ps_group]
                            nc.vector.tensor_tensor(
                                out=sl, in0=ps[:, :], in1=sl, op=mybir.AluOpType.min
                            )
                            nc.vector.tensor_reduce(
                                out=rt[:, g : g + 1],
                                in_=ps[:, :],
                                axis=mybir.AxisListType.X,
                                op=mybir.AluOpType.min,
                            )
                            if g == n_groups - 1:
                                nc.vector.tensor_reduce(
                                    out=rowp[:, i : i + 1],
                                    in_=rt[:, :],
                                    axis=mybir.AxisListType.X,
                                    op=mybir.AluOpType.min,
                                )
                        else:
                            nc.scalar.copy(
                                out=cp[:, g * ps_group : (g + 1) * ps_group],
                                in_=ps[:, :],
                            )
                            if fine:
                                sl = acc2[:, g * ps_group : (g + 1) * ps_group]
                                nc.vector.tensor_tensor(
                                    out=sl,
                                    in0=cp[:, g * ps_group : (g + 1) * ps_group],
                                    in1=sl,
                                    op=mybir.AluOpType.min,
                                )
                                if scr is None:
                                    scr = scr_pool.tile([P, half], st, tag="scr")
                                gh = ps_group // 2
                                lo = g * ps_group
                                nc.vector.tensor_tensor(
                                    out=scr[:, g * gh : (g + 1) * gh],
                                    in0=cp[:, lo : lo + gh],
                                    in1=cp[:, lo + gh : lo + ps_group],
                                    op=mybir.AluOpType.min,
                                )
                    if not direct:
                        if not fine:
                            # colmin over the whole staged row-tile, one wide op
                            nc.vector.tensor_tensor(
                                out=acc2[:, :],
                                in0=cp[:, :],
                                in1=acc2[:, :],
                                op=mybir.AluOpType.min,
                            )
                            if fused and _MINMIN is not None:
                                # rowmin in ONE fused custom-DVE pass
                                dummy = scr_pool.tile([P, 1], st, tag="dummy")
                                nc.vector._custom_dve(
                                    _MINMIN,
                                    out=dummy.broadcast_to(cp[:, :half].shape),
                                    in0=cp[:, :half],
                                    in1=cp[:, half:],
                                    s0=float(BIG),
                                    accum_out=rowp[:, i : i + 1],
                                )
                                continue
                            # rowmin: halving fold chain, in place in scr
                            scr = scr_pool.tile([P, half], st, tag="scr")
                            nc.vector.tensor_tensor(
                                out=scr[:, :],
                                in0=cp[:, :half],
                                in1=cp[:, half:],
                                op=mybir.AluOpType.min,
                            )
                            w = half // 2
                        else:
                            w = m // 4  # scr holds [P, m/2] of per-group L1 mins
                        while w >= 512:
                            nc.vector.tensor_tensor(
                                out=scr[:, :w],
                                in0=scr[:, :w],
                                in1=scr[:, w : 2 * w],
                                op=mybir.AluOpType.min,
                            )
                            w //= 2
                        nc.vector.tensor_reduce(
                            out=rowp[:, i : i + 1],
                            in_=scr[:, : 2 * w],
                            axis=mybir.AxisListType.X,
                            op=mybir.AluOpType.min,
                        )

            ident = singles.tile([P, P], st)
            make_identity(nc, ident[:, :])
            _emit_transposed(tc, nc, singles, rowp, ident, d1)
            _part_min_out(tc, nc, singles, acc2, ident, d2)

    nc.compile()
    return nc


def get_nc(n=N, m=M, reps=1, **kw):
    key = (n, m, reps, tuple(sorted(kw.items())))
    if key not in _NC_CACHE:
        _NC_CACHE[key] = build_nc(n, m, reps=reps, **kw)
    return _NC_CACHE[key]


def _split_hi_lo(a):
    hi = a.astype(BF16)
    lo = (a - hi.astype(np.float32)).astype(BF16)
    return hi, lo


def _stat_rows(u, u2h, u2l):
    """Stationary-side augmented rows [13, len] for points u [len, 3] f32."""
    uh, ul = _split_hi_lo(u)
    out = np.empty((K, u.shape[0]), BF16)
    out[0:3] = uh.T
    out[3:6] = uh.T
    out[6:9] = ul.T
    out[9] = BF16(1.0)
    out[10] = BF16(1.0)
    out[11] = u2h
    out[12] = u2l
    return np.ascontiguousarray(out)


def _mov_rows(v, v2h, v2l):
    """Moving-side augmented rows [13, len] for points v [len, 3] f32."""
    vh, vl = _split_hi_lo(v)
    out = np.empty((K, v.shape[0]), BF16)
    out[0:3] = (-2.0 * vh.astype(np.float32)).astype(BF16).T
    out[3:6] = (-2.0 * vl.astype(np.float32)).astype(BF16).T
    out[6:9] = out[0:3]
    out[9] = v2h
    out[10] = v2l
    out[11] = BF16(1.0)
    out[12] = BF16(1.0)
    return np.ascontiguousarray(out)


def _prep_core_inputs(x, y):
    """Augmented bf16 matrices for one batch: core computes d[n-tile, m] tiles
    with x stationary and y moving; both reductions happen in the same pass."""
    x = np.asarray(x, np.float32)
    y = np.asarray(y, np.float32)
    x2 = np.sum(x.astype(np.float64) * x, axis=-1).astype(np.float32)
    y2 = np.sum(y.astype(np.float64) * y, axis=-1).astype(np.float32)
    x2h, x2l = _split_hi_lo(x2)
    y2h, y2l = _split_hi_lo(y2)
    return {
        "sx": _stat_rows(x, x2h, x2l),
        "my": _mov_rows(y, y2h, y2l),
    }


def kernel(xyz1, xyz2):
    xyz1 = np.asarray(xyz1, np.float32)
    xyz2 = np.asarray(xyz2, np.float32)
    b, n, _ = xyz1.shape
    m = xyz2.shape[1]
    assert b == B and n == N and m == M, (b, n, m)

    nc = get_nc(n, m)
    in_maps = [_prep_core_inputs(xyz1[i], xyz2[i]) for i in range(b)]
    res = run_bass_kernel_spmd(nc, in_maps, core_ids=list(range(b)))
    dist1 = np.stack([res.results[i]["dist1"] for i in range(b)]).astype(np.float32)
    dist2 = np.stack([res.results[i]["dist2"] for i in range(b)]).astype(np.float32)
    return dist1, dist2


# revision 17
# speedup vs baseline: 1.6713x; 1.0636x over previous
"""Chamfer distance (L2) Bass kernel for 8 TRN2 NeuronCores.

Problem: xyz1 [B=8, N=8192, 3] f32, xyz2 [B=8, M=8192, 3] f32.
  d[b, n, m] = |xyz1[b,n] - xyz2[b,m]|^2
  dist1[b, n] = min_m d[b, n, m];  dist2[b, m] = min_n d[b, n, m]

Sharding: data-parallel over batch — core b handles batch b (B == n_cores == 8).
No collectives; outputs are gathered on the host.

Per-core algorithm — single pass over the distance matrix:
  d = x2[n] + y2[m] - 2*x.y is produced tile-by-tile by ONE TensorE matmul per
  output chunk via an augmented K=13 bf16 contraction (hi/lo splits of the
  coordinates for ~fp16-accurate cross terms, ones-rows folding the squared
  norms in), landing in fp32 PSUM groups of [128, 2048].

  Row-tiles are STAGED: ScalarE (the cheap PSUM reader) copies the four
  PSUM groups into one [128, 8192] fp16 SBUF tile cp, then per row-tile:
    - colmin:  acc2 = min(acc2, cp)            (one wide DVE tensor_tensor,
               2x all-16-bit mode; serial chain through acc2)
    - rowmin:  a log2 fold chain of in-place halving TT mins
               8192->4096->2048->1024->512 + one 1x tensor_reduce. The first
               (most expensive) fold runs on GPSIMD for most tiles, which
               balances the three engines; the DVE finishes the tail.
  (tensor_tensor_reduce would fuse the fold+reduce, but that ISA op is
  broken on this runtime — it wedges the NeuronCore.)

  dist1 comes straight from rowp; dist2's final min over the 128 partition
  lanes of acc2 uses PE transposes of 128x128 blocks + free-axis reduce_min.
"""

import sys

if "/opt/trn_rl_repo" not in sys.path:
    sys.path.insert(0, "/opt/trn_rl_repo")

import numpy as np
import ml_dtypes

import concourse.bass as bass  # noqa: F401
import concourse.mybir as mybir
import concourse.tile as tile
from concourse import bacc
from concourse.bass_utils import run_bass_kernel_spmd
from concourse.masks import make_identity


def _register_minmin_reduce():
    """Register a custom DVE op: out = min(in0, in1); accum_out =
    min(s0, min_k out[k]).  Fuses the rowmin fold layer-1 with the free-axis
    reduce — one DVE pass instead of a 4-op fold chain + reduce.  The op is
    appended to concourse.dve_ops.OPS at import (the per-NEFF DVE table is
    generated from that registry); the sha pin is self-computed."""
    import numpy as _np
    import concourse.dve_ops as dve_ops
    from concourse.dve_spec import C0, Spec, Src0, Src1, lower, minn
    from concourse.dve_uop import DveOpSpec

    name = "TT_MIN_MIN_REDUCE_ANT"
    for o in dve_ops.OPS:
        if o.name == name:
            return o

    def _ref(in0, in1, c0, c1, c2):
        b = _np.minimum(in0.astype(_np.float32), in1.astype(_np.float32))
        a = _np.minimum(
            _np.float32(c0), b.reshape(b.shape[0], -1).min(axis=-1, keepdims=True)
        )
        return b, a

    spec = Spec(body=minn(Src0, Src1), accum=minn, accum_init=C0, reference=_ref)
    row = max(dve_ops._SUB_OPCODE_FOR_NAME.values()) + 1
    assert row < 0x20
    shas = {}
    for ver in ("v3",):
        uops = lower(spec, ver=ver)
        shas[ver] = DveOpSpec(name=name, opcode=row, uops=uops, rd1_en=True).sha(ver)
    op = dve_ops.DveOp(name, spec, subdim=False, uops_sha=shas)
    dve_ops.OPS.append(op)
    dve_ops.CUSTOM_DVE_SPECS[name] = spec
    dve_ops._SUB_OPCODE_FOR_NAME[name] = row
    return op


try:
    _MINMIN = _register_minmin_reduce()
except Exception:  # pragma: no cover — fall back to the stock fold chain
    _MINMIN = None

BF16 = ml_dtypes.bfloat16

B = 8
N = 8192
M = 8192
P = 128  # output rows per tile (partition dim)
K = 13  # augmented contraction rows
BIG = 60000.0  # min-identity; finite in fp16, >> any squared distance here
ST = "float16"  # staging/accumulator dtype: 16-bit for DVE 2x mode, 2^-11 rounding

_NC_CACHE = {}


def _emit_transposed(tc, nc, pool, vec_sb, ident, out_dram):
    """vec_sb [P, n_blk] fp16 holds out[i*P + p] at [p, i]. PE-transpose to
    [n_blk, P], cast-copy to fp32, and DMA out contiguously (the direct
    [p, i]-strided DMA would scatter 4-byte elements)."""
    n_blk = vec_sb.shape[1]
    st = getattr(mybir.dt, ST)
    with tc.tile_pool(name="psum_o", bufs=1, space="PSUM") as psum_o:
        pt = psum_o.tile([n_blk, P], st)
        nc.tensor.transpose(pt[:, :], vec_sb[:, :], ident[:, :])
        ot = pool.tile([n_blk, P], mybir.dt.float32, tag="out_t")
        nc.vector.tensor_copy(ot[:, :], pt[:, :])
        nc.sync.dma_start(
            out=out_dram.ap().rearrange("(i p) -> i p", p=P), in_=ot[:, :]
        )


def _part_min_out(tc, nc, pool, acc, ident, out_dram):
    """Min over the 128 partitions of acc -> [m] via PE transpose + reduce."""
    m_len = acc.shape[1]
    n_blk = m_len // P
    st = getattr(mybir.dt, ST)
    osb = pool.tile([P, n_blk], st, tag="partmin_out")
    with tc.tile_pool(name="psum_t", bufs=2, space="PSUM") as psum_t:
        for t in range(n_blk):
            pst = psum_t.tile([P, P], st)
            nc.tensor.transpose(pst[:, :], acc[:, t * P : (t + 1) * P], ident[:, :])
            nc.vector.tensor_reduce(
                out=osb[:, t : t + 1],
                in_=pst[:, :],
                axis=mybir.AxisListType.X,
                op=mybir.AluOpType.min,
            )
    _emit_transposed(tc, nc, pool, osb, ident, out_dram)


def build_nc(
    n,
    m,
    mm_free=512,
    ps_group=2048,
    reps=1,
    direct_mod=0,
    gps_8=0,
    cp_bufs=2,
    scr_bufs=2,
    psum_bufs=2,
    fine=0,
    fused=1,
):
    """Build + compile the per-core Bass program (SPMD, same on all cores).

    direct_mod>0: every direct_mod-th row-tile skips ScalarE staging and is
    consumed from PSUM by the DVE (1x mode) — balances ACT vs DVE load.
    gps_8: of every 8 row-tiles, this many get their first rowmin fold done
    by GPSIMD instead of the DVE (engine load balancing).
    fine=1: per-PSUM-group colmin/fold ops instead of tile-wide ops.
    reps>1 repeats the main pass (identical results — min is idempotent);
    used only for timing: kernel time = slope of wall time vs reps.
    """
    ps_group = min(ps_group, m)
    mm_free = min(mm_free, ps_group)
    st = getattr(mybir.dt, ST)
    n_tiles = n // P
    n_groups = m // ps_group
    half = m // 2
    ghalf = ps_group // 2

    nc = bacc.Bacc("TRN2", target_bir_lowering=False, debug=False)
    sx = nc.dram_tensor("sx", [K, n], mybir.dt.bfloat16, kind="ExternalInput")
    my = nc.dram_tensor("my", [K, m], mybir.dt.bfloat16, kind="ExternalInput")
    d1 = nc.dram_tensor("dist1", [n], mybir.dt.float32, kind="ExternalOutput")
    d2 = nc.dram_tensor("dist2", [m], mybir.dt.float32, kind="ExternalOutput")

    with tile.TileContext(nc) as tc:
        with tc.tile_pool(name="singles", bufs=1) as singles:
            sx_sb = singles.tile([K, n], mybir.dt.bfloat16)
            my_sb = singles.tile([K, m], mybir.dt.bfloat16)
            nc.sync.dma_start(out=sx_sb[:, :], in_=sx.ap())
            nc.sync.dma_start(out=my_sb[:, :], in_=my.ap())

            acc2 = singles.tile([P, m], st)
            rowp = singles.tile([P, n_tiles], st)
            nc.vector.memset(acc2[:, :], BIG)

            with (
                tc.tile_pool(name="psum", bufs=psum_bufs, space="PSUM") as psum_pool,
                tc.tile_pool(name="cp", bufs=cp_bufs) as cp_pool,
                tc.tile_pool(name="scr", bufs=scr_bufs) as scr_pool,
            ):
                import contextlib

                rep_ctx = (
                    tc.For_i(0, reps, 1) if reps > 1 else contextlib.nullcontext()
                )
                with rep_ctx:
                  for i in range(n_tiles):
                    lhsT = sx_sb[:, i * P : (i + 1) * P]
                    direct = direct_mod > 0 and i % direct_mod == 0
                    cp = (
                        None
                        if direct
                        else cp_pool.tile([P, m], st, tag="cp")
                    )
                    if direct:
                        rt = scr_pool.tile([P, n_groups], st, tag="rtmp")
                    else:
                        rt = None
                    scr = None
                    for g in range(n_groups):
                        ps = psum_pool.tile([P, ps_group], mybir.dt.float32)
                        for t in range(ps_group // mm_free):
                            lo = g * ps_group + t * mm_free
                            nc.tensor.matmul(
                                ps[:, t * mm_free : (t + 1) * mm_free],
                                lhsT=lhsT,
                                rhs=my_sb[:, lo : lo + mm_free],
                                start=True,
                                stop=True,
                            )
                        if direct:
                            # DVE consumes PSUM at 1x — no ScalarE involved
                            sl = acc2[:, g * ps_group : (g + 1) * ps_group]
                            nc.vector.tensor_tensor(
                                out=sl, in0=ps[:, :], in1=sl, op=mybir.AluOpType.min
                            )
                            nc.vector.tensor_reduce(
                                out=rt[:, g : g + 1],
                                in_=ps[:, :],
                                axis=mybir.AxisListType.X,
                                op=mybir.AluOpType.min,
                            )
                            if g == n_groups - 1:
                                nc.vector.tensor_reduce(
                                    out=rowp[:, i : i + 1],
                                    in_=rt[:, :],
                                    axis=mybir.AxisListType.X,
                                    op=mybir.AluOpType.min,
                                )
                        else:
                            nc.scalar.copy(
                                out=cp[:, g * ps_group : (g + 1) * ps_group],
                                in_=ps[:, :],
                            )
                            if fine:
                                sl = acc2[:, g * ps_group : (g + 1) * ps_group]
                                nc.vector.tensor_tensor(
                                    out=sl,
                                    in0=cp[:, g * ps_group : (g + 1) * ps_group],
                                    in1=sl,
                                    op=mybir.AluOpType.min,
                                )
                                if scr is None:
                                    scr = scr_pool.tile([P, half], st, tag="scr")
                                gh = ps_group // 2
                                lo = g * ps_group
                                nc.vector.tensor_tensor(
                                    out=scr[:, g * gh : (g + 1) * gh],
                                    in0=cp[:, lo : lo + gh],
                                    in1=cp[:, lo + gh : lo + ps_group],
                                    op=mybir.AluOpType.min,
                                )
                    if not direct:
                        if not fine:
                            # colmin over the whole staged row-tile, one wide op
                            nc.vector.tensor_tensor(
                                out=acc2[:, :],
                                in0=cp[:, :],
                                in1=acc2[:, :],
                                op=mybir.AluOpType.min,
                            )
                            if fused and _MINMIN is not None:
                                # rowmin in ONE fused custom-DVE pass
                                dummy = scr_pool.tile([P, 1], st, tag="dummy")
                                nc.vector._custom_dve(
                                    _MINMIN,
                                    out=dummy.broadcast_to(cp[:, :half].shape),
                                    in0=cp[:, :half],
                                    in1=cp[:, half:],
                                    s0=float(BIG),
                                    accum_out=rowp[:, i : i + 1],
                                )
                                continue
                            # rowmin: halving fold chain, in place in scr
                            scr = scr_pool.tile([P, half], st, tag="scr")
                            nc.vector.tensor_tensor(
                                out=scr[:, :],
                                in0=cp[:, :half],
                                in1=cp[:, half:],
                                op=mybir.AluOpType.min,
                            )
                            w = half // 2
                        else:
                            w = m // 4  # scr holds [P, m/2] of per-group L1 mins
                        while w >= 512:
                            nc.vector.tensor_tensor(
                                out=scr[:, :w],
                                in0=scr[:, :w],
                                in1=scr[:, w : 2 * w],
                                op=mybir.AluOpType.min,
                            )
                            w //= 2
                        nc.vector.tensor_reduce(
                            out=rowp[:, i : i + 1],
                            in_=scr[:, : 2 * w],
                            axis=mybir.AxisListType.X,
                            op=mybir.AluOpType.min,
                        )

            ident = singles.tile([P, P], st)
            make_identity(nc, ident[:, :])
            _emit_transposed(tc, nc, singles, rowp, ident, d1)
            _part_min_out(tc, nc, singles, acc2, ident, d2)

    nc.compile()
    return nc


def get_nc(n=N, m=M, reps=1, **kw):
    key = (n, m, reps, tuple(sorted(kw.items())))
    if key not in _NC_CACHE:
        _NC_CACHE[key] = build_nc(n, m, reps=reps, **kw)
    return _NC_CACHE[key]


def _split_hi_lo(a):
    hi = a.astype(BF16)
    lo = (a - hi.astype(np.float32)).astype(BF16)
    return hi, lo


def _stat_rows(u, u2h, u2l):
    """Stationary-side augmented rows [13, len] for points u [len, 3] f32."""
    uh, ul = _split_hi_lo(u)
    out = np.empty((K, u.shape[0]), BF16)
    out[0:3] = uh.T
    out[3:6] = uh.T
    out[6:9] = ul.T
    out[9] = BF16(1.0)
    out[10] = BF16(1.0)
    out[11] = u2h
    out[12] = u2l
    return np.ascontiguousarray(out)


def _mov_rows(v, v2h, v2l):
    """Moving-side augmented rows [13, len] for points v [len, 3] f32."""
    vh, vl = _split_hi_lo(v)
    out = np.empty((K, v.shape[0]), BF16)
    out[0:3] = (-2.0 * vh.astype(np.float32)).astype(BF16).T
    out[3:6] = (-2.0 * vl.astype(np.float32)).astype(BF16).T
    out[6:9] = out[0:3]
    out[9] = v2h
    out[10] = v2l
    out[11] = BF16(1.0)
    out[12] = BF16(1.0)
    return np.ascontiguousarray(out)


def _prep_core_inputs(x, y):
    """Augmented bf16 matrices for one batch: core computes d[n-tile, m] tiles
    with x stationary and y moving; both reductions happen in the same pass."""
    x = np.asarray(x, np.float32)
    y = np.asarray(y, np.float32)
    x2 = np.sum(x.astype(np.float64) * x, axis=-1).astype(np.float32)
    y2 = np.sum(y.astype(np.float64) * y, axis=-1).astype(np.float32)
    x2h, x2l = _split_hi_lo(x2)
    y2h, y2l = _split_hi_lo(y2)
    return {
        "sx": _stat_rows(x, x2h, x2l),
        "my": _mov_rows(y, y2h, y2l),
    }


def kernel(xyz1, xyz2):
    xyz1 = np.asarray(xyz1, np.float32)
    xyz2 = np.asarray(xyz2, np.float32)
    b, n, _ = xyz1.shape
    m = xyz2.shape[1]
    assert b == B and n == N and m == M, (b, n, m)

    nc = get_nc(n, m)
    in_maps = [_prep_core_inputs(xyz1[i], xyz2[i]) for i in range(b)]
    res = run_bass_kernel_spmd(nc, in_maps, core_ids=list(range(b)))
    dist1 = np.stack([res.results[i]["dist1"] for i in range(b)]).astype(np.float32)
    dist2 = np.stack([res.results[i]["dist2"] for i in range(b)]).astype(np.float32)
    return dist1, dist2
